# revision 1
# baseline (speedup 1.0000x reference)
"""Trainium2 Bass kernel for AttnBlock (GroupNorm + 1x1-conv QKV self-attention
+ output proj + residual) on x: [4, 512, 64, 64] fp32, distributed over 8
NeuronCores.

Sharding: data-parallel over batch (4) x sequence-parallel over the N=H*W=4096
token axis (2 halves) = 8 cores. Each core receives the full image of its
batch element with the token axis rotated so that its 2048 query tokens come
first; it computes GroupNorm + K/V for all 4096 tokens (duplicated within the
batch pair -- no collectives needed) and Q/attention/output only for its 2048
queries. The host gathers the 8 [512, 2048] outputs back into [4, 512, 64, 64].

All matmuls run in bf16 on the PE array with fp32 PSUM accumulation; softmax
runs in fp32 (exp on the scalar engine straight out of PSUM). Key structure:
- GroupNorm is folded into the projections: wk@(s*x+t) = (wk*s)@x + (wk@t),
  so K/Q/V matmuls consume raw x tiles; the per-channel scale s lands in the
  weights (tiny DVE ops) and wk@t lands in the biases (tiny PE matmuls).
  This removes the normalized-activation stage entirely and its bf16 round.
- Scores are computed transposed (S^T = K^T Q per key tile) so softmax and
  the attention@V contraction need no transposes at all.
- The softmax 1/denominator is applied after the O-projection (it commutes
  with the linear projection), so the AV PSUM accumulators drain unnormalized
  in bf16 without waiting on the reciprocal chain.
- A 4-step score/exp lookahead across query blocks keeps the PE dense (and
  the HAM clock-gate at 2.4 GHz) through block boundaries.
- x ships in bf16 for the stats/projection path; the residual is added from a
  host-precomputed fp32 x+bo tensor.
Measured: ~379 us HW exec on 8 cores; max abs err ~3.9e-4 of the reference
absmax (rel l2 ~3.6e-4).
"""

import numpy as np
import ml_dtypes

B, C, H, W = 4, 512, 64, 64
N = H * W            # 4096 tokens
NQ = N // 2          # 2048 queries per core
P = 128              # partitions
CT = C // P          # 4 channel tiles
JT = N // P          # 32 key/token tiles
IBS = 512            # query block (free dim of score matmuls)
IB = NQ // IBS       # 4 query blocks per core
NCH = N // IBS       # 8 n-chunks for full-N projections
GROUPS = 32
GSIZE = C // GROUPS  # 16 channels per group
EPS = 1e-6
SM_SCALE = float(C) ** -0.5

N_CORES = 8

_cache = {}


def _build_nc():
    import concourse.bass as bass
    import concourse.mybir as mybir
    import concourse.tile as tile
    from concourse import bacc

    f32 = mybir.dt.float32
    bf16 = mybir.dt.bfloat16
    ID = mybir.ActivationFunctionType.Identity
    EXP = mybir.ActivationFunctionType.Exp
    SQRT = mybir.ActivationFunctionType.Sqrt

    nc = bacc.Bacc("TRN2")

    xr_d = nc.declare_dram_parameter("xr", [C, N], bf16, isOutput=False)
    w_d = {
        name: nc.declare_dram_parameter(name, [C, C], bf16, isOutput=False)
        for name in ("wqT", "wkT", "wvT", "woT")
    }
    cols_d = nc.declare_dram_parameter("cols", [C, 6], f32, isOutput=False)
    xqb_d = nc.declare_dram_parameter("xqb", [C, NQ], f32, isOutput=False)
    inda_d = nc.declare_dram_parameter("ind_a", [P, CT * GROUPS], bf16, isOutput=False)
    indb_d = nc.declare_dram_parameter("ind_b", [GROUPS, CT * P], bf16, isOutput=False)
    out_d = nc.declare_dram_parameter("out", [C, NQ], f32, isOutput=True)

    with tile.TileContext(nc) as tc:
        from contextlib import ExitStack

        with ExitStack() as ctx:
            const = ctx.enter_context(tc.tile_pool(name="const", bufs=1))
            pp_mm = ctx.enter_context(tc.tile_pool(name="pp_mm", bufs=3, space="PSUM"))
            pp_av = ctx.enter_context(tc.tile_pool(name="pp_av", bufs=4, space="PSUM"))
            pp_sm = ctx.enter_context(tc.tile_pool(name="pp_sm", bufs=1, space="PSUM"))

            # ---- batched small constants (few DMAs; issued after x) ----
            cols_t = [const.tile([P, 6], f32, tag=f"cols{t}", name=f"cols{t}")
                      for t in range(CT)]
            inda_t = const.tile([P, CT * GROUPS], bf16, tag="inda", name="inda")
            indb_t = const.tile([GROUPS, CT * P], bf16, tag="indb", name="indb")
            col_sb = {nm: [cols_t[t][:, i:i + 1] for t in range(CT)]
                      for i, nm in enumerate(("bq", "bk", "bv", "bo",
                                              "gamma", "beta"))}
            inda_sb = [inda_t[:, t * GROUPS:(t + 1) * GROUPS] for t in range(CT)]
            indb_sb = [indb_t[:, t * P:(t + 1) * P] for t in range(CT)]

            ones_colf = const.tile([P, 1], f32, tag="ones_colf", name="ones_colf")
            nc.vector.memset(ones_colf, 1.0)
            ones_rowf = const.tile([1, P], f32, tag="ones_rowf", name="ones_rowf")
            nc.vector.memset(ones_rowf, 1.0)

            stat_pool = ctx.enter_context(tc.tile_pool(name="stat", bufs=4 * CT))

            k_pool = ctx.enter_context(tc.tile_pool(name="k", bufs=CT))
            v_pool = ctx.enter_context(tc.tile_pool(name="v", bufs=JT))
            q_pool = ctx.enter_context(tc.tile_pool(name="q", bufs=CT))
            k_sb = [k_pool.tile([P, N], bf16, tag="k", name="k")
                    for _ in range(CT)]
            q_sb = [q_pool.tile([P, NQ], bf16, tag="q", name="q")
                    for _ in range(CT)]

            # ---- phase 1: x load (2 HW-DGE queues) + GroupNorm stats ----
            # stats for tiles 0,1 via DVE bn_stats; tiles 2,3 via ACT
            # Square/Identity with accum_out (free-dim sums) to halve the
            # serial DVE chain on the critical path.
            mv_sb = []
            with tc.tile_pool(name="xr", bufs=CT) as xr_pool:
                xr_sb = []
                st_sb = []
                acc_cols = []
                for t in range(CT):
                    xt = xr_pool.tile([P, N], bf16, tag="xr", name="xr")
                    xr_sb.append(xt)
                order = [(0, 0, nc.sync), (0, 1, nc.sync),
                         (0, 2, nc.scalar), (0, 3, nc.scalar),
                         (1, 0, nc.scalar), (1, 1, nc.scalar),
                         (2, 0, nc.sync), (2, 1, nc.sync),
                         (1, 2, nc.scalar), (1, 3, nc.scalar),
                         (2, 2, nc.sync), (2, 3, nc.sync),
                         (3, 0, nc.sync), (3, 2, nc.scalar),
                         (3, 1, nc.sync), (3, 3, nc.scalar)]
                for t, ch, eng in order:
                    csl = slice(ch * (N // 4), (ch + 1) * (N // 4))
                    eng.dma_start(out=xr_sb[t][:, csl],
                                  in_=xr_d[t * P:(t + 1) * P, csl])
                for t in range(CT):
                    xt = xr_sb[t]
                    xt_g = xt.rearrange("p (s f) -> p s f", f=512)
                    if t != 1:
                        st = stat_pool.tile([P, N // 512, 6], f32, tag="bnst",
                                            name="bnst")
                        sums = None
                        for s in range(N // 512):
                            nc.vector.bn_stats(out=st[:, s, :],
                                               in_=xt_g[:, s, :])
                    else:
                        st = None
                        sums = stat_pool.tile([P, 2, N // 512], f32, tag="acs",
                                              name="acs")
                        for s in range(N // 512):
                            scr = stat_pool.tile([P, 512], bf16, tag="scr",
                                                 name="scr", bufs=2)
                            nc.scalar.activation(
                                out=scr, in_=xt_g[:, s, :],
                                func=mybir.ActivationFunctionType.Square,
                                accum_out=sums[:, 1, s:s + 1])
                            nc.scalar.activation(
                                out=scr, in_=xt_g[:, s, :], func=ID,
                                accum_out=sums[:, 0, s:s + 1])
                    st_sb.append(st)
                    acc_cols.append(sums)

                # batched consts + weights + bv now (queues free after x)
                nc.sync.dma_start(out=inda_t, in_=inda_d[:, :])
                nc.sync.dma_start(out=indb_t, in_=indb_d[:, :])
                for t in range(CT):
                    nc.sync.dma_start(out=cols_t[t],
                                      in_=cols_d[t * P:(t + 1) * P, :])
                worig_cm = tc.tile_pool(name="worig", bufs=1)
                worig_pool = worig_cm.__enter__()
                w_sb = {}
                for name in ("wkT", "wqT", "wvT", "woT"):
                    tiles = []
                    for t in range(CT):
                        pool = const if name == "woT" else worig_pool
                        tw = pool.tile([P, C], bf16, tag=f"{name}{t}",
                                       name=f"{name}{t}")
                        nc.sync.dma_start(out=tw,
                                          in_=w_d[name][t * P:(t + 1) * P, :])
                        tiles.append(tw)
                    w_sb[name] = tiles
                bv_row = const.tile([1, C], f32, tag="bv_row", name="bv_row")
                nc.sync.dma_start(
                    out=bv_row,
                    in_=cols_d[:, 2:3].rearrange("c one -> one c"))

                for t in range(CT):
                    mv = stat_pool.tile([P, 2], f32, tag="mv", name="mv")
                    if st_sb[t] is not None:
                        nc.vector.bn_aggr(out=mv, in_=st_sb[t])
                        # mv = [mean, var] -> [mean, E[x^2]]
                        msq = stat_pool.tile([P, 1], f32, tag="msq", name="msq")
                        nc.vector.tensor_mul(msq, mv[:, 0:1], mv[:, 0:1])
                        nc.vector.tensor_add(mv[:, 1:2], mv[:, 1:2], msq)
                    else:
                        # sums[:, s, 0]=sum(x), [:, s, 1]=sum(x^2) per 512-chunk
                        sred = stat_pool.tile([P, 2], f32, tag="sred", name="sred")
                        nc.vector.tensor_reduce(
                            out=sred, in_=acc_cols[t],
                            op=mybir.AluOpType.add, axis=mybir.AxisListType.X)
                        nc.vector.tensor_scalar_mul(mv, sred, 1.0 / N)
                    mvb = stat_pool.tile([P, 2], bf16, tag="mvb", name="mvb")
                    nc.vector.tensor_copy(out=mvb, in_=mv)
                    mv_sb.append(mvb)

                # aggregate over channel groups: [32, 2] = [mean_g, E[x^2]_g]
                g_ps = pp_sm.tile([GROUPS, 2], f32, tag="den", name="den")
                for t in range(CT):
                    nc.tensor.matmul(g_ps, lhsT=inda_sb[t], rhs=mv_sb[t],
                                     start=(t == 0), stop=(t == CT - 1))
                g_sb = stat_pool.tile([GROUPS, 2], f32, tag="gsb", name="gsb")
                nc.vector.tensor_copy(out=g_sb, in_=g_ps)
                gm2 = stat_pool.tile([GROUPS, 1], f32, tag="gm2", name="gm2")
                nc.vector.tensor_mul(gm2, g_sb[:, 0:1], g_sb[:, 0:1])
                gvar = stat_pool.tile([GROUPS, 1], f32, tag="gvar", name="gvar")
                nc.vector.tensor_sub(gvar, g_sb[:, 1:2], gm2)
                eps_col = stat_pool.tile([GROUPS, 1], f32, tag="eps", name="eps")
                nc.vector.memset(eps_col, EPS)
                gstd = stat_pool.tile([GROUPS, 1], f32, tag="gstd", name="gstd")
                nc.scalar.activation(out=gstd, in_=gvar, func=SQRT, bias=eps_col)
                ga = stat_pool.tile([GROUPS, 1], f32, tag="ga", name="ga")
                nc.vector.reciprocal(out=ga, in_=gstd)
                coeffs = stat_pool.tile([GROUPS, 2], bf16, tag="coef", name="coef")
                nc.vector.tensor_copy(out=coeffs[:, 0:1], in_=ga)
                nc.vector.tensor_copy(out=coeffs[:, 1:2], in_=g_sb[:, 0:1])

                # broadcast group coeffs to per-channel scale/shift columns
                sc_cols = []
                tc_cols = []
                for t in range(CT):
                    b_ps = pp_sm.tile([P, 2], f32, tag="den", name="den")
                    nc.tensor.matmul(b_ps, lhsT=indb_sb[t], rhs=coeffs,
                                     start=True, stop=True)
                    bc = stat_pool.tile([P, 2], f32, tag="bc", name="bc")
                    nc.vector.tensor_copy(out=bc, in_=b_ps)
                    s_col = stat_pool.tile([P, 1], f32, tag="scol", name="scol")
                    nc.vector.tensor_mul(s_col, col_sb["gamma"][t], bc[:, 0:1])
                    tmp = stat_pool.tile([P, 1], f32, tag="tmp", name="tmp")
                    nc.vector.tensor_mul(tmp, bc[:, 1:2], s_col)
                    t_col = stat_pool.tile([P, 1], f32, tag="tcol", name="tcol")
                    nc.vector.tensor_sub(t_col, col_sb["beta"][t], tmp)
                    sc_cols.append(s_col)
                    tc_cols.append(t_col)

                # GroupNorm folding: wk@(s*x+t) = (wk*s)@x + wk@t.  Scale the
                # QKV weights per input channel on DVE; the wk@t bias
                # corrections are tiny PE matmuls (PE is idle here anyway).
                tcb = []
                for t in range(CT):
                    tb = stat_pool.tile([P, 1], bf16, tag="tcb", name="tcb")
                    nc.vector.tensor_copy(out=tb, in_=tc_cols[t])
                    tcb.append(tb)
                ws = {}
                for name in ("wkT", "wvT", "wqT"):
                    tiles = []
                    for ci in range(CT):
                        w2 = const.tile([P, C], bf16, tag=f"{name}s{ci}",
                                        name=f"{name}s{ci}")
                        if ci % 2 == 0:
                            nc.vector.tensor_scalar_mul(w2, w_sb[name][ci],
                                                        sc_cols[ci])
                        else:
                            nc.scalar.activation(out=w2, in_=w_sb[name][ci],
                                                 func=ID, scale=sc_cols[ci])
                        tiles.append(w2)
                    ws[name] = tiles

                # bias corrections: bk2[m] = bk[m] + sum_c wk[d,c] t_c
                bias2 = {}
                for name, bcol in (("wkT", "bk"), ("wqT", "bq")):
                    cols2 = []
                    for m in range(CT):
                        tk_ps = pp_sm.tile([P, 1], f32, tag="den", name="den")
                        for ci in range(CT):
                            nc.tensor.matmul(
                                tk_ps,
                                lhsT=w_sb[name][ci][:, m * P:(m + 1) * P],
                                rhs=tcb[ci],
                                start=(ci == 0), stop=(ci == CT - 1))
                        b2 = stat_pool.tile([P, 1], f32, tag=f"b2{name}{m}",
                                            name=f"b2{name}{m}")
                        nc.vector.tensor_scalar(
                            out=b2, in0=tk_ps, scalar1=col_sb[bcol][m],
                            scalar2=None, op0=mybir.AluOpType.add)
                        cols2.append(b2)
                    bias2[name] = cols2
                # v bias row: bvt[c] = bv[c] + sum_c' t_c' wv[c,c'], broadcast
                tv_ps = pp_sm.tile([1, C], f32, tag="den", name="den")
                for ci in range(CT):
                    nc.tensor.matmul(tv_ps, lhsT=tcb[ci], rhs=w_sb["wvT"][ci],
                                     start=(ci == 0), stop=(ci == CT - 1))
                bvt_row = stat_pool.tile([1, C], f32, tag="bvtr", name="bvtr")
                nc.vector.tensor_add(bvt_row, tv_ps, bv_row)
                bvt_ps = pp_av.tile([P, IBS], f32, tag="pav", name="bvtps")
                nc.tensor.matmul(bvt_ps, lhsT=ones_rowf, rhs=bvt_row,
                                 start=True, stop=True)
                bvt_bcast = const.tile([P, C], f32, tag="bvt_bcast",
                                       name="bvt_bcast")
                nc.scalar.activation(out=bvt_bcast, in_=bvt_ps, func=ID)
                worig_cm.__exit__(None, None, None)

                # ---- phase 2: projections straight from x ----
                for nch in range(NCH):
                    hsl = slice(nch * IBS, (nch + 1) * IBS)
                    for m in range(CT):
                        ps = pp_mm.tile([P, IBS], f32, tag="mm", name="mm")
                        for ci in range(CT):
                            nc.tensor.matmul(
                                ps,
                                lhsT=ws["wkT"][ci][:, m * P:(m + 1) * P],
                                rhs=xr_sb[ci][:, hsl],
                                start=(ci == 0), stop=(ci == CT - 1))
                        nc.scalar.activation(
                            out=k_sb[m][:, hsl], in_=ps,
                            func=ID, bias=bias2["wkT"][m], scale=1.0)

                for nch in range(IB):
                    hsl = slice(nch * IBS, (nch + 1) * IBS)
                    for m in range(CT):
                        ps = pp_mm.tile([P, IBS], f32, tag="mm", name="mm")
                        for ci in range(CT):
                            nc.tensor.matmul(
                                ps,
                                lhsT=ws["wqT"][ci][:, m * P:(m + 1) * P],
                                rhs=xr_sb[ci][:, hsl],
                                start=(ci == 0), stop=(ci == CT - 1))
                        nc.scalar.activation(
                            out=q_sb[m][:, hsl], in_=ps,
                            func=ID, bias=bias2["wqT"][m], scale=1.0)

                # V^T projection; bias-add on DVE drains each PSUM right away
                v_sb = []
                for jt in range(JT):
                    ps = pp_mm.tile([P, IBS], f32, tag="mm", name="mm")
                    for ci in range(CT):
                        nc.tensor.matmul(
                            ps,
                            lhsT=xr_sb[ci][:, jt * P:(jt + 1) * P],
                            rhs=ws["wvT"][ci],
                            start=(ci == 0), stop=(ci == CT - 1))
                    vt = v_pool.tile([P, C], bf16, tag="v", name="v")
                    nc.vector.tensor_add(vt, ps, bvt_bcast)
                    v_sb.append(vt)

            # ---- phase 3: attention + output proj + residual ----
            p_pool = ctx.enter_context(tc.tile_pool(name="p", bufs=8))
            xqb_pool = ctx.enter_context(tc.tile_pool(name="xqb", bufs=3))
            a_pool = ctx.enter_context(tc.tile_pool(name="a", bufs=2 * CT))
            o_pool = ctx.enter_context(tc.tile_pool(name="o", bufs=3))
            sm_pool = ctx.enter_context(tc.tile_pool(name="sm", bufs=2))

            LOOKAHEAD = 4

            def emit_scores(ib, jt):
                isl = slice(ib * IBS, (ib + 1) * IBS)
                ps = pp_mm.tile([P, IBS], f32, tag="mm", name="mm")
                for ci in range(CT):
                    nc.tensor.matmul(
                        ps,
                        lhsT=k_sb[ci][:, jt * P:(jt + 1) * P],
                        rhs=q_sb[ci][:, isl],
                        start=(ci == 0), stop=(ci == CT - 1))
                pt = p_pool.tile([P, IBS], bf16, tag="p", name="p")
                nc.scalar.activation(out=pt, in_=ps, func=EXP, scale=SM_SCALE)
                return pt

            pending = {}
            for ib in range(IB):
                isl = slice(ib * IBS, (ib + 1) * IBS)
                pav = [pp_av.tile([P, IBS], f32, tag="pav", name="pav")
                       for _ in range(CT)]
                acc = sm_pool.tile([P, IBS], f32, tag="acc", name="acc")
                accg = sm_pool.tile([P, IBS], f32, tag="accg", name="accg")
                for jt in range(JT):
                    pt = pending.pop((ib, jt), None)
                    if pt is None:
                        pt = emit_scores(ib, jt)
                    # softmax denominator partials, split DVE/GPSIMD
                    if jt == 0:
                        nc.vector.tensor_copy(out=acc, in_=pt)
                    elif jt == 1:
                        nc.gpsimd.tensor_copy(out=accg, in_=pt)
                    elif jt % 2 == 0:
                        nc.vector.tensor_add(acc, acc, pt)
                    else:
                        nc.gpsimd.tensor_add(accg, accg, pt)
                    for m in range(CT):
                        nc.tensor.matmul(pav[m],
                                         lhsT=v_sb[jt][:, m * P:(m + 1) * P],
                                         rhs=pt,
                                         start=(jt == 0), stop=(jt == JT - 1))

                # unnormalized attention output -> bf16 (frees pav banks
                # fast); emitted BEFORE the lookahead so the drains don't queue
                # behind the lookahead exps on ACT. The 1/den scale commutes
                # past the linear O-projection.
                a_sb = []
                for m in range(CT):
                    at = a_pool.tile([P, IBS], bf16, tag="a", name="a")
                    nc.scalar.activation(out=at, in_=pav[m], func=ID)
                    a_sb.append(at)

                # score lookahead into the next block keeps the PE busy while
                # the denominator/reciprocal tail of this block resolves
                if ib + 1 < IB:
                    for la in range(LOOKAHEAD):
                        pending[(ib + 1, la)] = emit_scores(ib + 1, la)

                nc.vector.tensor_add(acc, acc, accg)
                # den[i] = sum_p acc[p, i]  (partition reduce, tiny fp32 matmul)
                den_ps = pp_sm.tile([1, IBS], f32, tag="den", name="den")
                nc.tensor.matmul(den_ps, lhsT=ones_colf, rhs=acc,
                                 start=True, stop=True)
                recip_row = sm_pool.tile([1, IBS], f32, tag="recip_row",
                                         name="recip_row")
                nc.vector.reciprocal(out=recip_row, in_=den_ps)

                po_l = []
                xqb_l = []
                for dt_ in range(CT):
                    xqb_t = xqb_pool.tile([P, IBS], f32, tag="xqb", name="xqb")
                    nc.sync.dma_start(out=xqb_t,
                                      in_=xqb_d[dt_ * P:(dt_ + 1) * P, isl])
                    po = pp_mm.tile([P, IBS], f32, tag="mm", name="mm")
                    for m in range(CT):
                        nc.tensor.matmul(
                            po,
                            lhsT=w_sb["woT"][m][:, dt_ * P:(dt_ + 1) * P],
                            rhs=a_sb[m],
                            start=(m == 0), stop=(m == CT - 1))
                    po_l.append(po)
                    xqb_l.append(xqb_t)

                # broadcast 1/den across partitions with a K=1 fp32 matmul
                bc_ps = pp_av.tile([P, IBS], f32, tag="pav", name="bcps")
                nc.tensor.matmul(bc_ps, lhsT=ones_rowf, rhs=recip_row,
                                 start=True, stop=True)
                recip_b = sm_pool.tile([P, IBS], f32, tag="recip_b",
                                       name="recip_b")
                nc.scalar.activation(out=recip_b, in_=bc_ps, func=ID)

                for dt_ in range(CT):
                    o1 = o_pool.tile([P, IBS], f32, tag="o1", name="o1")
                    nc.vector.tensor_mul(o1, po_l[dt_], recip_b)
                    o2 = o_pool.tile([P, IBS], f32, tag="o2", name="o2")
                    nc.vector.tensor_add(o2, o1, xqb_l[dt_])
                    nc.sync.dma_start(out=out_d[dt_ * P:(dt_ + 1) * P, isl],
                                      in_=o2)

    nc.finalize()
    return nc


def _make_consts():
    """Constant (core-independent) input arrays (packed)."""
    ind_a = np.zeros((P, CT * GROUPS), ml_dtypes.bfloat16)
    ind_b = np.zeros((GROUPS, CT * P), ml_dtypes.bfloat16)
    for t in range(CT):
        for p in range(P):
            g = (t * P + p) // GSIZE
            ind_a[p, t * GROUPS + g] = 1.0 / GSIZE
            ind_b[g - 8 * t if False else g, t * P + p] = 1.0
    return ind_a, ind_b


def make_in_maps(x, gn_gamma, gn_beta, wq, bq, wk, bk, wv, bv, wo, bo):
    ind_a, ind_b = _make_consts()
    bf = ml_dtypes.bfloat16
    cols = np.stack([np.asarray(a, np.float32) for a in
                     (bq, bk, bv, bo, gn_gamma, gn_beta)], axis=1)
    common = {
        "wqT": np.ascontiguousarray(np.asarray(wq, np.float32).T).astype(bf),
        "wkT": np.ascontiguousarray(np.asarray(wk, np.float32).T).astype(bf),
        "wvT": np.ascontiguousarray(np.asarray(wv, np.float32).T).astype(bf),
        "woT": np.ascontiguousarray(np.asarray(wo, np.float32).T).astype(bf),
        "cols": np.ascontiguousarray(cols),
        "ind_a": ind_a,
        "ind_b": ind_b,
    }
    x = np.asarray(x, np.float32)
    in_maps = []
    for core in range(N_CORES):
        b, half = divmod(core, 2)
        xb = x[b].reshape(C, N)
        xr = np.concatenate(
            [xb[:, half * NQ:(half + 1) * NQ],
             xb[:, (1 - half) * NQ:(2 - half) * NQ]],
            axis=1)
        xqb = xr[:, :NQ] + np.asarray(bo, np.float32).reshape(C, 1)
        in_maps.append({"xr": np.ascontiguousarray(xr).astype(bf),
                        "xqb": np.ascontiguousarray(xqb), **common})
    return in_maps


def gather_out(results):
    out = np.empty((B, C, N), np.float32)
    for core in range(N_CORES):
        b, half = divmod(core, 2)
        out[b][:, half * NQ:(half + 1) * NQ] = results[core]["out"]
    return out.reshape(B, C, H, W)


def get_nc():
    if "nc" not in _cache:
        _cache["nc"] = _build_nc()
    return _cache["nc"]


def kernel(**inputs):
    from concourse.bass_utils import run_bass_kernel_spmd

    nc = get_nc()
    in_maps = make_in_maps(**inputs)
    res = run_bass_kernel_spmd(nc, in_maps, list(range(N_CORES)))
    return gather_out(res.results)


if __name__ == "__main__":
    nc = _build_nc()
    print("built ok:", len(nc.m.functions[0].allocations), "allocations")



# revision 8
# speedup vs baseline: 1.3926x; 1.3926x over previous
"""Trainium2 Bass kernel for AttnBlock (GroupNorm + 1x1-conv QKV self-attention
+ output proj + residual) on x: [4, 512, 64, 64] fp32, distributed over 8
NeuronCores.

Sharding: data-parallel over batch (4) x sequence-parallel over the N=H*W=4096
token axis (2 halves) = 8 cores. Each core receives the full image of its
batch element with the token axis rotated so that its 2048 query tokens come
first; it computes GroupNorm + K/V for all 4096 tokens (duplicated within the
batch pair -- no collectives needed) and Q/attention/output only for its 2048
queries. The host gathers the 8 [512, 2048] outputs back into [4, 512, 64, 64].

All matmuls run in bf16 on the PE array with fp32 PSUM accumulation; softmax
runs in fp32 (exp on the scalar engine straight out of PSUM). Key structure:
- GroupNorm is folded into the projections: wk@(s*x+t) = (wk*s)@x + (wk@t),
  so K/Q/V matmuls consume raw x tiles; the per-channel scale s lands in the
  weights (tiny DVE ops) and wk@t lands in the biases (tiny PE matmuls).
  This removes the normalized-activation stage entirely and its bf16 round.
- Scores are computed transposed (S^T = K^T Q per key tile) so softmax and
  the attention@V contraction need no transposes at all.
- The softmax 1/denominator is applied after the O-projection (it commutes
  with the linear projection), so the AV PSUM accumulators drain unnormalized
  in bf16 without waiting on the reciprocal chain.
- A 4-step score/exp lookahead across query blocks keeps the PE dense (and
  the HAM clock-gate at 2.4 GHz) through block boundaries.
- x ships in bf16 for the stats/projection path; the residual is added from a
  host-precomputed fp32 x+bo tensor.
Measured: ~379 us HW exec on 8 cores; max abs err ~3.9e-4 of the reference
absmax (rel l2 ~3.6e-4).
"""

import numpy as np
import ml_dtypes

B, C, H, W = 4, 512, 64, 64
N = H * W            # 4096 tokens
NQ = N // 2          # 2048 queries per core
P = 128              # partitions
CT = C // P          # 4 channel tiles
JT = N // P          # 32 key/token tiles
IBS = 512            # query block (free dim of score matmuls)
IB = NQ // IBS       # 4 query blocks per core
NCH = N // IBS       # 8 n-chunks for full-N projections
GROUPS = 32
GSIZE = C // GROUPS  # 16 channels per group
EPS = 1e-6
SM_SCALE = float(C) ** -0.5

N_CORES = 8

_cache = {}


def _build_nc():
    import concourse.bass as bass
    import concourse.mybir as mybir
    import concourse.tile as tile
    from concourse import bacc

    f32 = mybir.dt.float32
    bf16 = mybir.dt.bfloat16
    ID = mybir.ActivationFunctionType.Identity
    EXP = mybir.ActivationFunctionType.Exp
    SQRT = mybir.ActivationFunctionType.Sqrt

    nc = bacc.Bacc("TRN2")

    xr_d = nc.declare_dram_parameter("xr", [C, N], bf16, isOutput=False)
    w_d = {
        name: nc.declare_dram_parameter(name, [C, C], bf16, isOutput=False)
        for name in ("wqT", "wkT", "wvT", "woT")
    }
    cols_d = nc.declare_dram_parameter("cols", [C, 6], f32, isOutput=False)
    xqb_d = nc.declare_dram_parameter("xqb", [C, NQ], f32, isOutput=False)
    inda_d = nc.declare_dram_parameter("ind_a", [P, CT * GROUPS], bf16, isOutput=False)
    indb_d = nc.declare_dram_parameter("ind_b", [GROUPS, CT * P], bf16, isOutput=False)
    out_d = nc.declare_dram_parameter("out", [C, NQ], f32, isOutput=True)

    with tile.TileContext(nc) as tc:
        from contextlib import ExitStack

        with ExitStack() as ctx:
            const = ctx.enter_context(tc.tile_pool(name="const", bufs=1))
            pp_mm = ctx.enter_context(tc.tile_pool(name="pp_mm", bufs=3, space="PSUM"))
            pp_av = ctx.enter_context(tc.tile_pool(name="pp_av", bufs=4, space="PSUM"))
            pp_sm = ctx.enter_context(tc.tile_pool(name="pp_sm", bufs=1, space="PSUM"))

            # ---- batched small constants (few DMAs; issued after x) ----
            cols_t = [const.tile([P, 6], f32, tag=f"cols{t}", name=f"cols{t}")
                      for t in range(CT)]
            inda_t = const.tile([P, CT * GROUPS], bf16, tag="inda", name="inda")
            indb_t = const.tile([GROUPS, CT * P], bf16, tag="indb", name="indb")
            col_sb = {nm: [cols_t[t][:, i:i + 1] for t in range(CT)]
                      for i, nm in enumerate(("bq", "bk", "bv", "bo",
                                              "gamma", "beta"))}
            inda_sb = [inda_t[:, t * GROUPS:(t + 1) * GROUPS] for t in range(CT)]
            indb_sb = [indb_t[:, t * P:(t + 1) * P] for t in range(CT)]

            ones_colf = const.tile([P, 1], f32, tag="ones_colf", name="ones_colf")
            nc.vector.memset(ones_colf, 1.0)
            ones_rowf = const.tile([1, P], f32, tag="ones_rowf", name="ones_rowf")
            nc.vector.memset(ones_rowf, 1.0)

            stat_pool = ctx.enter_context(tc.tile_pool(name="stat", bufs=4 * CT))

            fp8 = mybir.dt.float8e4
            DR = mybir.MatmulPerfMode.DoubleRow
            CP = CT // 2         # 2 channel pair-tiles
            JP = JT // 2         # 16 key pair-tiles
            ESH = 2.5            # exp shift: p = exp(s*scale - ESH)

            k_pool = ctx.enter_context(tc.tile_pool(name="k", bufs=CP))
            v_pool = ctx.enter_context(tc.tile_pool(name="v", bufs=JP))
            q_pool = ctx.enter_context(tc.tile_pool(name="q", bufs=CP))
            k_sb = [k_pool.tile([P, 2, N], fp8, tag="k", name="k")
                    for _ in range(CP)]
            q_sb = [q_pool.tile([P, 2, NQ], fp8, tag="q", name="q")
                    for _ in range(CP)]

            # ---- phase 1: x load (2 HW-DGE queues) + GroupNorm stats ----
            # stats for tiles 0,1 via DVE bn_stats; tiles 2,3 via ACT
            # Square/Identity with accum_out (free-dim sums) to halve the
            # serial DVE chain on the critical path.
            mv_sb = []
            with tc.tile_pool(name="xr", bufs=CT) as xr_pool:
                xr_sb = []
                st_sb = []
                acc_cols = []
                for t in range(CT):
                    xt = xr_pool.tile([P, N], bf16, tag="xr", name="xr")
                    xr_sb.append(xt)
                order = [(0, 0, nc.sync), (0, 1, nc.sync),
                         (0, 2, nc.scalar), (0, 3, nc.scalar),
                         (1, 0, nc.scalar), (1, 1, nc.scalar),
                         (2, 0, nc.sync), (2, 1, nc.sync),
                         (1, 2, nc.scalar), (1, 3, nc.scalar),
                         (2, 2, nc.sync), (2, 3, nc.sync),
                         (3, 0, nc.sync), (3, 2, nc.scalar),
                         (3, 1, nc.sync), (3, 3, nc.scalar)]
                for t, ch, eng in order:
                    csl = slice(ch * (N // 4), (ch + 1) * (N // 4))
                    eng.dma_start(out=xr_sb[t][:, csl],
                                  in_=xr_d[t * P:(t + 1) * P, csl])
                for t in range(CT):
                    xt = xr_sb[t]
                    xt_g = xt.rearrange("p (s f) -> p s f", f=512)
                    if t != 1:
                        st = stat_pool.tile([P, N // 512, 6], f32, tag="bnst",
                                            name="bnst")
                        sums = None
                        for s in range(N // 512):
                            nc.vector.bn_stats(out=st[:, s, :],
                                               in_=xt_g[:, s, :])
                    else:
                        st = None
                        sums = stat_pool.tile([P, 2, N // 512], f32, tag="acs",
                                              name="acs")
                        for s in range(N // 512):
                            scr = stat_pool.tile([P, 512], bf16, tag="scr",
                                                 name="scr", bufs=2)
                            nc.scalar.activation(
                                out=scr, in_=xt_g[:, s, :],
                                func=mybir.ActivationFunctionType.Square,
                                accum_out=sums[:, 1, s:s + 1])
                            nc.scalar.activation(
                                out=scr, in_=xt_g[:, s, :], func=ID,
                                accum_out=sums[:, 0, s:s + 1])
                    st_sb.append(st)
                    acc_cols.append(sums)

                # batched consts + weights + bv now (queues free after x)
                nc.sync.dma_start(out=inda_t, in_=inda_d[:, :])
                nc.sync.dma_start(out=indb_t, in_=indb_d[:, :])
                for t in range(CT):
                    nc.sync.dma_start(out=cols_t[t],
                                      in_=cols_d[t * P:(t + 1) * P, :])
                worig_cm = tc.tile_pool(name="worig", bufs=1)
                worig_pool = worig_cm.__enter__()
                w_sb = {}
                for name in ("wkT", "wqT", "wvT", "woT"):
                    tiles = []
                    for t in range(CT):
                        pool = const if name == "woT" else worig_pool
                        tw = pool.tile([P, C], bf16, tag=f"{name}{t}",
                                       name=f"{name}{t}")
                        nc.sync.dma_start(out=tw,
                                          in_=w_d[name][t * P:(t + 1) * P, :])
                        tiles.append(tw)
                    w_sb[name] = tiles
                bv_row = const.tile([1, C], f32, tag="bv_row", name="bv_row")
                nc.sync.dma_start(
                    out=bv_row,
                    in_=cols_d[:, 2:3].rearrange("c one -> one c"))

                for t in range(CT):
                    mv = stat_pool.tile([P, 2], f32, tag="mv", name="mv")
                    if st_sb[t] is not None:
                        nc.vector.bn_aggr(out=mv, in_=st_sb[t])
                        # mv = [mean, var] -> [mean, E[x^2]]
                        msq = stat_pool.tile([P, 1], f32, tag="msq", name="msq")
                        nc.vector.tensor_mul(msq, mv[:, 0:1], mv[:, 0:1])
                        nc.vector.tensor_add(mv[:, 1:2], mv[:, 1:2], msq)
                    else:
                        # sums[:, s, 0]=sum(x), [:, s, 1]=sum(x^2) per 512-chunk
                        sred = stat_pool.tile([P, 2], f32, tag="sred", name="sred")
                        nc.vector.tensor_reduce(
                            out=sred, in_=acc_cols[t],
                            op=mybir.AluOpType.add, axis=mybir.AxisListType.X)
                        nc.vector.tensor_scalar_mul(mv, sred, 1.0 / N)
                    mvb = stat_pool.tile([P, 2], bf16, tag="mvb", name="mvb")
                    nc.vector.tensor_copy(out=mvb, in_=mv)
                    mv_sb.append(mvb)

                # aggregate over channel groups: [32, 2] = [mean_g, E[x^2]_g]
                g_ps = pp_sm.tile([GROUPS, 2], f32, tag="den", name="den")
                for t in range(CT):
                    nc.tensor.matmul(g_ps, lhsT=inda_sb[t], rhs=mv_sb[t],
                                     start=(t == 0), stop=(t == CT - 1))
                g_sb = stat_pool.tile([GROUPS, 2], f32, tag="gsb", name="gsb")
                nc.vector.tensor_copy(out=g_sb, in_=g_ps)
                gm2 = stat_pool.tile([GROUPS, 1], f32, tag="gm2", name="gm2")
                nc.vector.tensor_mul(gm2, g_sb[:, 0:1], g_sb[:, 0:1])
                gvar = stat_pool.tile([GROUPS, 1], f32, tag="gvar", name="gvar")
                nc.vector.tensor_sub(gvar, g_sb[:, 1:2], gm2)
                eps_col = stat_pool.tile([GROUPS, 1], f32, tag="eps", name="eps")
                nc.vector.memset(eps_col, EPS)
                gstd = stat_pool.tile([GROUPS, 1], f32, tag="gstd", name="gstd")
                nc.scalar.activation(out=gstd, in_=gvar, func=SQRT, bias=eps_col)
                ga = stat_pool.tile([GROUPS, 1], f32, tag="ga", name="ga")
                nc.vector.reciprocal(out=ga, in_=gstd)
                coeffs = stat_pool.tile([GROUPS, 2], bf16, tag="coef", name="coef")
                nc.vector.tensor_copy(out=coeffs[:, 0:1], in_=ga)
                nc.vector.tensor_copy(out=coeffs[:, 1:2], in_=g_sb[:, 0:1])

                # broadcast group coeffs to per-channel scale/shift columns
                sc_cols = []
                tc_cols = []
                for t in range(CT):
                    b_ps = pp_sm.tile([P, 2], f32, tag="den", name="den")
                    nc.tensor.matmul(b_ps, lhsT=indb_sb[t], rhs=coeffs,
                                     start=True, stop=True)
                    bc = stat_pool.tile([P, 2], f32, tag="bc", name="bc")
                    nc.vector.tensor_copy(out=bc, in_=b_ps)
                    s_col = stat_pool.tile([P, 1], f32, tag="scol", name="scol")
                    nc.vector.tensor_mul(s_col, col_sb["gamma"][t], bc[:, 0:1])
                    tmp = stat_pool.tile([P, 1], f32, tag="tmp", name="tmp")
                    nc.vector.tensor_mul(tmp, bc[:, 1:2], s_col)
                    t_col = stat_pool.tile([P, 1], f32, tag="tcol", name="tcol")
                    nc.vector.tensor_sub(t_col, col_sb["beta"][t], tmp)
                    sc_cols.append(s_col)
                    tc_cols.append(t_col)

                # GroupNorm folding: wk@(s*x+t) = (wk*s)@x + wk@t.  Scale the
                # QKV weights per input channel on DVE; the wk@t bias
                # corrections are tiny PE matmuls (PE is idle here anyway).
                tcb = []
                for t in range(CT):
                    tb = stat_pool.tile([P, 1], bf16, tag="tcb", name="tcb")
                    nc.vector.tensor_copy(out=tb, in_=tc_cols[t])
                    tcb.append(tb)
                ws = {}
                for name in ("wkT", "wvT", "wqT"):
                    tiles = []
                    for ci in range(CT):
                        w2 = const.tile([P, C], bf16, tag=f"{name}s{ci}",
                                        name=f"{name}s{ci}")
                        if ci % 2 == 0:
                            nc.vector.tensor_scalar_mul(w2, w_sb[name][ci],
                                                        sc_cols[ci])
                        else:
                            nc.scalar.activation(out=w2, in_=w_sb[name][ci],
                                                 func=ID, scale=sc_cols[ci])
                        tiles.append(w2)
                    ws[name] = tiles

                # bias corrections: bk2[m] = bk[m] + sum_c wk[d,c] t_c
                bias2 = {}
                for name, bcol in (("wkT", "bk"), ("wqT", "bq")):
                    cols2 = []
                    for m in range(CT):
                        tk_ps = pp_sm.tile([P, 1], f32, tag="den", name="den")
                        for ci in range(CT):
                            nc.tensor.matmul(
                                tk_ps,
                                lhsT=w_sb[name][ci][:, m * P:(m + 1) * P],
                                rhs=tcb[ci],
                                start=(ci == 0), stop=(ci == CT - 1))
                        b2 = stat_pool.tile([P, 1], f32, tag=f"b2{name}{m}",
                                            name=f"b2{name}{m}")
                        nc.vector.tensor_scalar(
                            out=b2, in0=tk_ps, scalar1=col_sb[bcol][m],
                            scalar2=None, op0=mybir.AluOpType.add)
                        cols2.append(b2)
                    bias2[name] = cols2
                # v bias row: bvt[c] = bv[c] + sum_c' t_c' wv[c,c'], broadcast
                tv_ps = pp_sm.tile([1, C], f32, tag="den", name="den")
                for ci in range(CT):
                    nc.tensor.matmul(tv_ps, lhsT=tcb[ci], rhs=w_sb["wvT"][ci],
                                     start=(ci == 0), stop=(ci == CT - 1))
                bvt_row = stat_pool.tile([1, C], f32, tag="bvtr", name="bvtr")
                nc.vector.tensor_add(bvt_row, tv_ps, bv_row)
                bvt_ps = pp_av.tile([P, IBS], f32, tag="pav", name="bvtps")
                nc.tensor.matmul(bvt_ps, lhsT=ones_rowf, rhs=bvt_row,
                                 start=True, stop=True)
                bvt_bcast = const.tile([P, C], f32, tag="bvt_bcast",
                                       name="bvt_bcast")
                nc.scalar.activation(out=bvt_bcast, in_=bvt_ps, func=ID)
                worig_cm.__exit__(None, None, None)

                # ---- phase 2: projections straight from x ----
                for nch in range(NCH):
                    hsl = slice(nch * IBS, (nch + 1) * IBS)
                    for m in range(CT):
                        ps = pp_mm.tile([P, IBS], f32, tag="mm", name="mm")
                        for ci in range(CT):
                            nc.tensor.matmul(
                                ps,
                                lhsT=ws["wkT"][ci][:, m * P:(m + 1) * P],
                                rhs=xr_sb[ci][:, hsl],
                                start=(ci == 0), stop=(ci == CT - 1))
                        nc.scalar.activation(
                            out=k_sb[m // 2][:, m % 2, hsl], in_=ps,
                            func=ID, bias=bias2["wkT"][m], scale=1.0)

                for nch in range(IB):
                    hsl = slice(nch * IBS, (nch + 1) * IBS)
                    for m in range(CT):
                        ps = pp_mm.tile([P, IBS], f32, tag="mm", name="mm")
                        for ci in range(CT):
                            nc.tensor.matmul(
                                ps,
                                lhsT=ws["wqT"][ci][:, m * P:(m + 1) * P],
                                rhs=xr_sb[ci][:, hsl],
                                start=(ci == 0), stop=(ci == CT - 1))
                        nc.scalar.activation(
                            out=q_sb[m // 2][:, m % 2, hsl], in_=ps,
                            func=ID, bias=bias2["wqT"][m], scale=1.0)

                # V^T projection; bias-add on DVE drains each PSUM right away
                v_sb = [v_pool.tile([P, 2, C], fp8, tag="v", name="v")
                        for _ in range(JP)]
                for jt in range(JT):
                    ps = pp_mm.tile([P, IBS], f32, tag="mm", name="mm")
                    for ci in range(CT):
                        nc.tensor.matmul(
                            ps,
                            lhsT=xr_sb[ci][:, jt * P:(jt + 1) * P],
                            rhs=ws["wvT"][ci],
                            start=(ci == 0), stop=(ci == CT - 1))
                    nc.vector.tensor_add(v_sb[jt // 2][:, jt % 2, :],
                                         ps, bvt_bcast)

            # ---- phase 3: attention + output proj + residual ----
            p_pool = ctx.enter_context(tc.tile_pool(name="p", bufs=8))
            xqb_pool = ctx.enter_context(tc.tile_pool(name="xqb", bufs=3))
            a_pool = ctx.enter_context(tc.tile_pool(name="a", bufs=2 * CT))
            o_pool = ctx.enter_context(tc.tile_pool(name="o", bufs=3))
            sm_pool = ctx.enter_context(tc.tile_pool(name="sm", bufs=2))

            LOOKAHEAD = 2   # pair-steps (= 4 key tiles)

            ones_colb = const.tile([P, 1], bf16, tag="ones_colb",
                                   name="ones_colb")
            nc.vector.memset(ones_colb, 1.0)
            esh_col = const.tile([P, 1], f32, tag="esh_col", name="esh_col")
            nc.vector.memset(esh_col, -ESH)
            ones_rowb = const.tile([1, P], bf16, tag="ones_rowb",
                                   name="ones_rowb")
            nc.vector.memset(ones_rowb, 1.0)

            def emit_scores_pair(ib, jp):
                # S^T for key tiles (2*jp, 2*jp+1): fp8 DoubleRow matmuls,
                # exp'd (with global shift) into a [P, 2, IBS] fp8 pair tile.
                isl = slice(ib * IBS, (ib + 1) * IBS)
                pt = p_pool.tile([P, 2, IBS], fp8, tag="p", name="p")
                for jj in range(2):
                    jt = 2 * jp + jj
                    ps = pp_mm.tile([P, IBS], f32, tag="mm", name="mm")
                    for t in range(CP):
                        nc.tensor.matmul(
                            ps,
                            lhsT=k_sb[t][:, :, jt * P:(jt + 1) * P],
                            rhs=q_sb[t][:, :, isl],
                            start=(t == 0), stop=(t == CP - 1),
                            perf_mode=DR)
                    nc.scalar.activation(out=pt[:, jj, :], in_=ps, func=EXP,
                                         scale=SM_SCALE, bias=esh_col)
                return pt

            pending = {}
            for ib in range(IB):
                isl = slice(ib * IBS, (ib + 1) * IBS)
                pav = [pp_av.tile([P, IBS], f32, tag="pav", name="pav")
                       for _ in range(CT)]
                acc = sm_pool.tile([P, 2 * IBS], f32, tag="acc", name="acc")
                accg = sm_pool.tile([P, 2 * IBS], f32, tag="accg", name="accg")
                for jp in range(JP):
                    pt = pending.pop((ib, jp), None)
                    if pt is None:
                        pt = emit_scores_pair(ib, jp)
                    ptv = pt.rearrange("p two f -> p (two f)")
                    # softmax denominator partials, split DVE/GPSIMD
                    if jp == 0:
                        nc.vector.tensor_copy(out=acc, in_=ptv)
                    elif jp == 1:
                        nc.gpsimd.tensor_copy(out=accg, in_=ptv)
                    elif jp % 2 == 0:
                        nc.vector.tensor_add(acc, acc, ptv)
                    else:
                        nc.gpsimd.tensor_add(accg, accg, ptv)
                    for m in range(CT):
                        nc.tensor.matmul(pav[m],
                                         lhsT=v_sb[jp][:, :,
                                                       m * P:(m + 1) * P],
                                         rhs=pt,
                                         start=(jp == 0), stop=(jp == JP - 1),
                                         perf_mode=DR)

                # unnormalized attention output -> bf16 (frees pav banks
                # fast); emitted BEFORE the lookahead so the drains don't queue
                # behind the lookahead exps on ACT. The 1/den scale commutes
                # past the linear O-projection.
                a_sb = []
                for m in range(CT):
                    at = a_pool.tile([P, IBS], bf16, tag="a", name="a")
                    nc.scalar.activation(out=at, in_=pav[m], func=ID)
                    a_sb.append(at)

                # score lookahead into the next block keeps the PE busy while
                # the denominator/reciprocal tail of this block resolves
                if ib + 1 < IB:
                    for la in range(LOOKAHEAD):
                        pending[(ib + 1, la)] = emit_scores_pair(ib + 1, la)

                nc.vector.tensor_add(acc, acc, accg)
                accb = sm_pool.tile([P, 2 * IBS], bf16, tag="accb",
                                    name="accb")
                nc.vector.tensor_copy(out=accb, in_=acc)
                # den[i] = sum over partitions and both halves (bf16 matmuls)
                den_ps = pp_sm.tile([1, IBS], f32, tag="den", name="den")
                nc.tensor.matmul(den_ps, lhsT=ones_colb, rhs=accb[:, 0:IBS],
                                 start=True, stop=False)
                nc.tensor.matmul(den_ps, lhsT=ones_colb,
                                 rhs=accb[:, IBS:2 * IBS],
                                 start=False, stop=True)
                recip_row = sm_pool.tile([1, IBS], bf16, tag="recip_row",
                                         name="recip_row")
                with nc.allow_low_precision(reason="1/den to bf16 is plenty"):
                    nc.vector.reciprocal(out=recip_row, in_=den_ps)

                po_l = []
                xqb_l = []
                for dt_ in range(CT):
                    xqb_t = xqb_pool.tile([P, IBS], f32, tag="xqb", name="xqb")
                    nc.sync.dma_start(out=xqb_t,
                                      in_=xqb_d[dt_ * P:(dt_ + 1) * P, isl])
                    po = pp_mm.tile([P, IBS], f32, tag="mm", name="mm")
                    for m in range(CT):
                        nc.tensor.matmul(
                            po,
                            lhsT=w_sb["woT"][m][:, dt_ * P:(dt_ + 1) * P],
                            rhs=a_sb[m],
                            start=(m == 0), stop=(m == CT - 1))
                    po_l.append(po)
                    xqb_l.append(xqb_t)

                # broadcast 1/den across partitions with a K=1 bf16 matmul
                bc_ps = pp_av.tile([P, IBS], f32, tag="pav", name="bcps")
                nc.tensor.matmul(bc_ps, lhsT=ones_rowb, rhs=recip_row,
                                 start=True, stop=True)
                recip_b = sm_pool.tile([P, IBS], f32, tag="recip_b",
                                       name="recip_b")
                nc.scalar.activation(out=recip_b, in_=bc_ps, func=ID)

                for dt_ in range(CT):
                    o1 = o_pool.tile([P, IBS], f32, tag="o1", name="o1")
                    nc.vector.tensor_mul(o1, po_l[dt_], recip_b)
                    o2 = o_pool.tile([P, IBS], f32, tag="o2", name="o2")
                    nc.vector.tensor_add(o2, o1, xqb_l[dt_])
                    nc.sync.dma_start(out=out_d[dt_ * P:(dt_ + 1) * P, isl],
                                      in_=o2)

    nc.finalize()
    return nc


def _make_consts():
    """Constant (core-independent) input arrays (packed)."""
    ind_a = np.zeros((P, CT * GROUPS), ml_dtypes.bfloat16)
    ind_b = np.zeros((GROUPS, CT * P), ml_dtypes.bfloat16)
    for t in range(CT):
        for p in range(P):
            g = (t * P + p) // GSIZE
            ind_a[p, t * GROUPS + g] = 1.0 / GSIZE
            ind_b[g - 8 * t if False else g, t * P + p] = 1.0
    return ind_a, ind_b


def make_in_maps(x, gn_gamma, gn_beta, wq, bq, wk, bk, wv, bv, wo, bo):
    ind_a, ind_b = _make_consts()
    bf = ml_dtypes.bfloat16
    cols = np.stack([np.asarray(a, np.float32) for a in
                     (bq, bk, bv, bo, gn_gamma, gn_beta)], axis=1)
    common = {
        "wqT": np.ascontiguousarray(np.asarray(wq, np.float32).T).astype(bf),
        "wkT": np.ascontiguousarray(np.asarray(wk, np.float32).T).astype(bf),
        "wvT": np.ascontiguousarray(np.asarray(wv, np.float32).T).astype(bf),
        "woT": np.ascontiguousarray(np.asarray(wo, np.float32).T).astype(bf),
        "cols": np.ascontiguousarray(cols),
        "ind_a": ind_a,
        "ind_b": ind_b,
    }
    x = np.asarray(x, np.float32)
    in_maps = []
    for core in range(N_CORES):
        b, half = divmod(core, 2)
        xb = x[b].reshape(C, N)
        xr = np.concatenate(
            [xb[:, half * NQ:(half + 1) * NQ],
             xb[:, (1 - half) * NQ:(2 - half) * NQ]],
            axis=1)
        xqb = xr[:, :NQ] + np.asarray(bo, np.float32).reshape(C, 1)
        in_maps.append({"xr": np.ascontiguousarray(xr).astype(bf),
                        "xqb": np.ascontiguousarray(xqb), **common})
    return in_maps


def gather_out(results):
    out = np.empty((B, C, N), np.float32)
    for core in range(N_CORES):
        b, half = divmod(core, 2)
        out[b][:, half * NQ:(half + 1) * NQ] = results[core]["out"]
    return out.reshape(B, C, H, W)


def get_nc():
    if "nc" not in _cache:
        _cache["nc"] = _build_nc()
    return _cache["nc"]


def kernel(**inputs):
    from concourse.bass_utils import run_bass_kernel_spmd

    nc = get_nc()
    in_maps = make_in_maps(**inputs)
    res = run_bass_kernel_spmd(nc, in_maps, list(range(N_CORES)))
    return gather_out(res.results)


if __name__ == "__main__":
    nc = _build_nc()
    print("built ok:", len(nc.m.functions[0].allocations), "allocations")

